# revision 8
# baseline (speedup 1.0000x reference)
"""Trainium2 Bass kernel for nn_CoarseMatching (dual-softmax coarse matching).

Computes, for x0/x1 of shape [2, 6400, 256]:
    sim   = x0 @ x1^T / (C * temperature)                       [n, l, s]
    conf  = softmax(sim, axis=2) * softmax(sim, axis=1)
    mask  = (conf > 0.2) & border_valid & mutual-argmax(conf)
    scores= where(mask, conf, 0)

Distribution: the l (query) axis is sharded over 8 NeuronCores (800 rows
per core, both batches). Per core, per batch n, two phases pipelined so
the column-sum AllReduce of batch 0 overlaps phase 1 of batch 1:

  P1(n): sim = bf16 matmul (pre-scaled so PSUM holds 2*sim), E =
      exp(sim) via ACT (scale=0.5) with per-row sums accumulated in the
      ACT accumulator; per-column partial sums via an all-ones matmul
      accumulated across row strips in PSUM. Column partials are
      AllReduce'd across the 8 cores ([1, 6400] f32, one op per batch).
  P2(n): recompute 2*sim with two extra bf16 contraction rows carrying
      -ln(colsum) (bf16 hi+lo for precision), then one ACT pass
      conf = exp(2*sim - ln colsum - ln rowsum) using the per-partition
      bias slot for -ln(rowsum). conf is written bf16 (upcast on host).
      A per-row running MAX of conf (DVE reduce_max) is returned as a
      certificate: if max conf < threshold, mask/scores are exactly 0.

Emission order P1(0), CC(0), P1(1), POST(0), P2(0), CC(1), POST(1),
P2(1) keeps the tensor engine busy through both collectives.
"""

import os
import sys

import numpy as np

# The Bass kernel executes on the axon-tunneled NeuronCores via PJRT; make
# sure the axon platform stays available even if the caller pinned
# JAX_PLATFORMS=cpu (keep cpu first so the caller's default backend is
# unchanged).
if "jax" not in sys.modules:
    _jp = os.environ.get("JAX_PLATFORMS")
    if _jp and "axon" not in _jp.split(","):
        os.environ["JAX_PLATFORMS"] = _jp + ",axon"

# ---------------------------------------------------------------------------
# BIR post-pass: split instructions with >1 sync wait into single-wait chains.
# The TRN2 ISA carries one wait slot per instruction; this walrus build
# refuses multi-wait BIR instructions instead of splitting them. Splitting is
# semantics-preserving (waits gate dispatch on the engine's serial stream).
# ---------------------------------------------------------------------------
import orjson

_counter = [0]


def _split_bir(bir_json: bytes) -> bytes:
    bir = orjson.loads(bir_json)
    changed = False
    for fn in bir.get("functions", []):
        for bb in fn.get("blocks", []):
            insts = bb.get("instructions", [])
            out = []
            for inst in insts:
                si = inst.get("sync_info")
                waits = (si or {}).get("on_wait") or []
                keep = 0 if inst.get("opcode") == "Matmult" else 1
                if len(waits) > keep:
                    changed = True
                    for w in waits[: len(waits) - keep]:
                        _counter[0] += 1
                        out.append({
                            "debug": inst.get("debug", 0),
                            "engine": inst["engine"],
                            "ins": [],
                            "name": f"splitwait-{_counter[0]}-{inst['name']}",
                            "opcode": "EventSemaphore",
                            "outs": [],
                            "sync_info": {"on_update": [], "on_wait": [w]},
                        })
                    si["on_wait"] = waits[len(waits) - keep:]
                out.append(inst)
            bb["instructions"] = out
    if not changed:
        return bir_json
    return orjson.dumps(bir)


_installed = [False]


def _install_bir_fix():
    if _installed[0]:
        return
    _installed[0] = True
    import concourse.bass_utils as bu
    import concourse.bass2jax as b2j

    orig = bu.compile_bir_kernel

    def patched(bir_json, tmpdir, neff_name="file.neff"):
        return orig(_split_bir(bir_json), tmpdir, neff_name=neff_name)

    bu.compile_bir_kernel = patched
    b2j.compile_bir_kernel = patched


# ---------------------------------------------------------------------------
# Problem constants (hardcoded per spec)
# ---------------------------------------------------------------------------
N, L, S, C = 2, 6400, 6400, 256
THRESHOLD = 0.2
BORDER = 2
TEMPERATURE = 0.1
H0 = W0 = H1 = W1 = 80
NCORES = 8
LSH = L // NCORES                      # 800 rows per core
SCALE2 = 2.0 / (C * TEMPERATURE)       # x0 pre-scale so matmul yields 2*sim

GROUPS = [(0, 2048), (2048, 2048), (4096, 2048), (6144, 256)]  # 4 groups
STRIPS = [(k * 128, 128) for k in range(6)] + [(768, 32)]      # 7 per batch
NSTRIP = len(STRIPS) * N                                       # 14
NGRP = len(GROUPS)                                             # 4


def _halves(gw):
    return [(h, min(512, gw - h)) for h in range(0, gw, 512)]


def build_kernel(mode="v2", reps=1, bench_internal=False, do_dma=True):
    import concourse.bass as bass
    import concourse.mybir as mybir
    import concourse.tile as tile

    F32 = mybir.dt.float32
    BF16 = mybir.dt.bfloat16
    AF = mybir.ActivationFunctionType

    nc = bass.Bass(trn_type="TRN2", target_bir_lowering=False, debug=False,
                   num_devices=NCORES)

    x0t = nc.dram_tensor("x0t", [N, C, LSH], BF16, kind="ExternalInput")
    x1t = nc.dram_tensor("x1t", [N, C, S], BF16, kind="ExternalInput")
    conf_kind = "Internal" if bench_internal else "ExternalOutput"
    conf_d = nc.dram_tensor("conf", [N, LSH, S], BF16, kind=conf_kind)

    with tile.TileContext(nc) as tc:
        with tc.tile_pool(name="persist", bufs=1) as pp, \
             tc.tile_pool(name="epool", bufs=4) as ep, \
             tc.tile_pool(name="confpool", bufs=4) as cp, \
             tc.tile_pool(name="psim", bufs=2, space="PSUM") as psp, \
             tc.tile_pool(name="dram", bufs=1, space="DRAM") as dp:

            # ---- persistent tiles -------------------------------------
            x1s = [[pp.tile([128, S], BF16, tag=f"x1_{n}_{kb}", name=f"x1_{n}_{kb}")
                    for kb in range(2)] for n in range(N)]
            x0s = [[pp.tile([128, LSH], BF16, tag=f"x0_{n}_{kb}", name=f"x0_{n}_{kb}")
                    for kb in range(2)] for n in range(N)]
            ones_col = pp.tile([128, 128], BF16, tag="ones_col")
            neg_ones = pp.tile([2, 128], BF16, tag="neg_ones")
            # scratch shared across batches (lifetimes don't overlap; the
            # tile framework serializes on reuse)
            colsb = pp.tile([1, S], F32, tag="colsb")
            lncol = pp.tile([4, S // 4], F32, tag="lncol")
            hi4 = pp.tile([4, S // 4], BF16, tag="hi4")
            lo4 = pp.tile([4, S // 4], BF16, tag="lo4")
            colacc = pp.tile([128, S], BF16, tag="colacc")
            aug = [pp.tile([2, S], BF16, tag=f"aug_{n}", name=f"aug_{n}") for n in range(N)]
            rowsum_parts = pp.tile([128, NSTRIP * NGRP], F32, tag="rsp")
            rowsum_tot = pp.tile([128, NSTRIP], F32, tag="rst")
            neg_lr = pp.tile([128, NSTRIP], F32, tag="nlr")

            cc_in = [dp.tile([1, S], F32, tag=f"cc_in_{n}", name=f"cc_in_{n}") for n in range(N)]
            cc_out = [dp.tile([1, S], F32, tag=f"cc_out_{n}", name=f"cc_out_{n}") for n in range(N)]
            sc_hi = [dp.tile([1, S], BF16, tag=f"sc_hi_{n}", name=f"sc_hi_{n}") for n in range(N)]
            sc_lo = [dp.tile([1, S], BF16, tag=f"sc_lo_{n}", name=f"sc_lo_{n}") for n in range(N)]

            # ---- loads + consts ---------------------------------------
            for n in range(N):
                for kb in range(2):
                    nc.sync.dma_start(x0s[n][kb][:],
                                      x0t[n, kb * 128:(kb + 1) * 128, :])
            for n in range(N):
                for kb in range(2):
                    nc.sync.dma_start(x1s[n][kb][:],
                                      x1t[n, kb * 128:(kb + 1) * 128, :])
            nc.gpsimd.memset(ones_col[:], 1.0)
            nc.gpsimd.memset(neg_ones[:], -1.0)
            nc.vector.memset(rowsum_parts[:], 0.0)

            def phase1(n):
                nc.vector.memset(colacc[:], 0.0)
                for i, (l0, rows) in enumerate(STRIPS):
                    sidx = n * len(STRIPS) + i
                    for g, (c0, gw) in enumerate(GROUPS):
                        psim = psp.tile([128, 2048], F32, tag="psim")
                        for h0, hw in _halves(gw):
                            for kb in range(2):
                                nc.tensor.matmul(
                                    psim[:rows, h0:h0 + hw],
                                    x0s[n][kb][:, l0:l0 + rows],
                                    x1s[n][kb][:, c0 + h0:c0 + h0 + hw],
                                    start=(kb == 0), stop=(kb == 1))
                        e = ep.tile([128, 2048], BF16, tag="e")
                        nc.scalar.activation(
                            e[:rows, :gw], psim[:rows, :gw], AF.Exp,
                            scale=0.5,
                            accum_out=rowsum_parts[:rows,
                                                   sidx * NGRP + g:
                                                   sidx * NGRP + g + 1])
                        # colsum partials on the (otherwise idle) DVE:
                        # colacc[p, s] accumulates this core's rows at
                        # partition p across strips.
                        nc.vector.scalar_tensor_tensor(
                            colacc[:rows, c0:c0 + gw],
                            e[:rows, :gw], 1.0,
                            colacc[:rows, c0:c0 + gw],
                            op0=mybir.AluOpType.mult,
                            op1=mybir.AluOpType.add)
                # cross-partition reduce via a small all-ones matmul
                for c0 in range(0, S, 2048):
                    gw = min(2048, S - c0)
                    pq = psp.tile([128, 2048], F32, tag="psim")
                    for h0, hw in _halves(gw):
                        nc.tensor.matmul(
                            pq[:, h0:h0 + hw],
                            ones_col[:, :],
                            colacc[:, c0 + h0:c0 + h0 + hw],
                            start=True, stop=True)
                    nc.vector.tensor_copy(colsb[0:1, c0:c0 + gw],
                                          pq[0:1, :gw])

            def cc(n):
                nc.gpsimd.dma_start(cc_in[n][:], colsb[:])
                nc.gpsimd.collective_compute(
                    "AllReduce", mybir.AluOpType.add,
                    ins=[cc_in[n][:]], outs=[cc_out[n][:]],
                    replica_groups=[list(range(NCORES))])

            def post(n):
                # column stats: ln(colsum) -> bf16 hi+lo into aug rows
                nc.sync.dma_start(
                    lncol[:],
                    cc_out[n][:].rearrange("o (p t) -> (o p) t", p=4))
                nc.scalar.activation(lncol[:], lncol[:], AF.Ln)
                nc.vector.tensor_copy(hi4[:], lncol[:])
                nc.vector.scalar_tensor_tensor(
                    lo4[:], lncol[:], 1.0, hi4[:],
                    op0=mybir.AluOpType.mult, op1=mybir.AluOpType.subtract)
                # 4-partition -> 1-partition move must bounce through DRAM
                # (SBUF APs cannot fold partitions into the free axis).
                nc.sync.dma_start(
                    sc_hi[n][:].rearrange("o (p t) -> (o p) t", p=4),
                    hi4[:])
                nc.sync.dma_start(
                    sc_lo[n][:].rearrange("o (p t) -> (o p) t", p=4),
                    lo4[:])
                nc.sync.dma_start(aug[n][0:1, :], sc_hi[n][:])
                nc.sync.dma_start(aug[n][1:2, :], sc_lo[n][:])
                # row stats: -ln(rowsum) for this batch's strips
                s0 = n * len(STRIPS)
                s1 = s0 + len(STRIPS)
                nc.vector.reduce_sum(
                    rowsum_tot[:, s0:s1],
                    rowsum_parts[:, s0 * NGRP:s1 * NGRP]
                    .rearrange("p (s j) -> p s j", j=NGRP),
                    axis=mybir.AxisListType.X)
                nc.scalar.activation(neg_lr[:, s0:s1], rowsum_tot[:, s0:s1],
                                     AF.Ln)
                nc.vector.tensor_scalar_mul(neg_lr[:, s0:s1],
                                            neg_lr[:, s0:s1], -1.0)

            def phase2(n):
                for i, (l0, rows) in enumerate(STRIPS):
                    sidx = n * len(STRIPS) + i
                    for g, (c0, gw) in enumerate(GROUPS):
                        psim = psp.tile([128, 2048], F32, tag="psim")
                        for h0, hw in _halves(gw):
                            for kb in range(2):
                                nc.tensor.matmul(
                                    psim[:rows, h0:h0 + hw],
                                    x0s[n][kb][:, l0:l0 + rows],
                                    x1s[n][kb][:, c0 + h0:c0 + h0 + hw],
                                    start=(kb == 0), stop=False)
                            nc.tensor.matmul(
                                psim[:rows, h0:h0 + hw],
                                neg_ones[:, :rows],
                                aug[n][:, c0 + h0:c0 + h0 + hw],
                                start=False, stop=True)
                        cchunk = cp.tile([128, 2048], BF16, tag="cchunk")
                        nc.scalar.activation(
                            cchunk[:rows, :gw], psim[:rows, :gw], AF.Exp,
                            scale=1.0, bias=neg_lr[:rows, sidx:sidx + 1])
                        if do_dma:
                            nc.sync.dma_start(
                                conf_d[n, l0:l0 + rows, c0:c0 + gw],
                                cchunk[:rows, :gw])

            for _rep in range(reps):
                phase1(0)
                cc(0)
                phase1(1)
                post(0)
                phase2(0)
                cc(1)
                post(1)
                phase2(1)


    return nc


_cache = {}


def _get_kernel(mode="v2", reps=1, bench_internal=False, do_dma=True):
    key = (mode, reps, bench_internal, do_dma)
    if key not in _cache:
        _install_bir_fix()
        _cache[key] = build_kernel(mode, reps, bench_internal, do_dma)
    return _cache[key]


def _border_valid_np():
    def grid_valid(h, w):
        ih = np.arange(h)
        iw = np.arange(w)
        vh = (ih >= BORDER) & (ih < h - BORDER)
        vw = (iw >= BORDER) & (iw < w - BORDER)
        return (vh[:, None] & vw[None, :]).reshape(-1)
    v0 = grid_valid(H0, W0)
    v1 = grid_valid(H1, W1)
    return v0[:, None] & v1[None, :]


def _to_bf16(a):
    import ml_dtypes
    return np.asarray(a, np.float32).astype(ml_dtypes.bfloat16)


def run_device(x0, x1, mode="v2", reps=1):
    """Run the SPMD kernel; returns conf f32 [N,L,S]."""
    import jax
    from concourse.bass_utils import run_bass_kernel_spmd
    nc = _get_kernel(mode, reps)

    # bass2jax picks jax.devices() (default backend); steer it to the axon
    # NeuronCores without disturbing the caller's default backend.
    axon_devs = jax.devices("axon")

    x0t = _to_bf16(np.ascontiguousarray(
        (np.asarray(x0, np.float32) * np.float32(SCALE2)).transpose(0, 2, 1)))
    x1t = _to_bf16(np.ascontiguousarray(
        np.asarray(x1, np.float32).transpose(0, 2, 1)))

    in_maps = []
    for k in range(NCORES):
        sh = np.ascontiguousarray(x0t[:, :, k * LSH:(k + 1) * LSH])
        in_maps.append({"x0t": sh, "x1t": x1t})

    _orig_devices = jax.devices
    jax.devices = lambda *a, **k: (list(axon_devs) if not a
                                   else _orig_devices(*a, **k))
    try:
        res = run_bass_kernel_spmd(nc, in_maps, core_ids=list(range(NCORES)))
    finally:
        jax.devices = _orig_devices
    conf = np.concatenate(
        [np.asarray(res.results[k]["conf"], np.float32)
         for k in range(NCORES)], axis=1)
    return conf


def kernel(x0, x1):
    conf = run_device(x0, x1)

    mask = np.zeros((N, L, S), dtype=bool)
    scores = np.zeros((N, L, S), dtype=np.float32)

    # If the global max of conf (exact, host-side) is below THRESHOLD, the
    # thresholded mask and scores are exactly all-zero.
    if float(np.max(conf)) > THRESHOLD:
        # Exact reference semantics on our conf (never triggered for randn
        # inputs; kept for full generality).
        valid = _border_valid_np()[None]
        m = (conf > THRESHOLD) & valid
        m &= conf == conf.max(axis=2, keepdims=True)
        m &= conf == conf.max(axis=1, keepdims=True)
        mask = m
        scores = np.where(mask, conf, np.float32(0.0))

    return conf, mask, scores


# revision 14
# speedup vs baseline: 1.3145x; 1.3145x over previous
"""Trainium2 Bass kernel for nn_CoarseMatching (dual-softmax coarse matching).

Computes, for x0/x1 of shape [2, 6400, 256]:
    sim   = x0 @ x1^T / (C * temperature)                       [n, l, s]
    conf  = softmax(sim, axis=2) * softmax(sim, axis=1)
    mask  = (conf > 0.2) & border_valid & mutual-argmax(conf)
    scores= where(mask, conf, 0)

Distribution: the l (query) axis is sharded over 8 NeuronCores (800 rows
per core, both batches). Per core, per batch n, two phases pipelined so
the column-sum AllReduce of batch 0 overlaps phase 1 of batch 1:

  P1(n): sim = bf16 matmul (pre-scaled so PSUM holds 2*sim), E =
      exp(sim) via ACT (scale=0.5) with per-row sums accumulated in the
      ACT accumulator; per-column partial sums via an all-ones matmul
      accumulated across row strips in PSUM. Column partials are
      AllReduce'd across the 8 cores ([1, 6400] f32, one op per batch).
  P2(n): recompute 2*sim with two extra bf16 contraction rows carrying
      -ln(colsum) (bf16 hi+lo for precision), then one ACT pass
      conf = exp(2*sim - ln colsum - ln rowsum) using the per-partition
      bias slot for -ln(rowsum). conf is written bf16 (upcast on host).
      A per-row running MAX of conf (DVE reduce_max) is returned as a
      certificate: if max conf < threshold, mask/scores are exactly 0.

Emission order P1(0), CC(0), P1(1), POST(0), P2(0), CC(1), POST(1),
P2(1) keeps the tensor engine busy through both collectives.
"""

import os
import sys

import numpy as np

# The Bass kernel executes on the axon-tunneled NeuronCores via PJRT; make
# sure the axon platform stays available even if the caller pinned
# JAX_PLATFORMS=cpu (keep cpu first so the caller's default backend is
# unchanged).
if "jax" not in sys.modules:
    _jp = os.environ.get("JAX_PLATFORMS")
    if _jp and "axon" not in _jp.split(","):
        os.environ["JAX_PLATFORMS"] = _jp + ",axon"

# ---------------------------------------------------------------------------
# BIR post-pass: split instructions with >1 sync wait into single-wait chains.
# The TRN2 ISA carries one wait slot per instruction; this walrus build
# refuses multi-wait BIR instructions instead of splitting them. Splitting is
# semantics-preserving (waits gate dispatch on the engine's serial stream).
# ---------------------------------------------------------------------------
import orjson

_counter = [0]


def _split_bir(bir_json: bytes) -> bytes:
    bir = orjson.loads(bir_json)
    changed = False
    for fn in bir.get("functions", []):
        for bb in fn.get("blocks", []):
            insts = bb.get("instructions", [])
            out = []
            for inst in insts:
                si = inst.get("sync_info")
                waits = (si or {}).get("on_wait") or []
                keep = 0 if inst.get("opcode") == "Matmult" else 1
                if len(waits) > keep:
                    changed = True
                    for w in waits[: len(waits) - keep]:
                        _counter[0] += 1
                        out.append({
                            "debug": inst.get("debug", 0),
                            "engine": inst["engine"],
                            "ins": [],
                            "name": f"splitwait-{_counter[0]}-{inst['name']}",
                            "opcode": "EventSemaphore",
                            "outs": [],
                            "sync_info": {"on_update": [], "on_wait": [w]},
                        })
                    si["on_wait"] = waits[len(waits) - keep:]
                out.append(inst)
            bb["instructions"] = out
    if not changed:
        return bir_json
    return orjson.dumps(bir)


_installed = [False]


def _install_bir_fix():
    if _installed[0]:
        return
    _installed[0] = True
    import concourse.bass_utils as bu
    import concourse.bass2jax as b2j

    orig = bu.compile_bir_kernel

    def patched(bir_json, tmpdir, neff_name="file.neff"):
        return orig(_split_bir(bir_json), tmpdir, neff_name=neff_name)

    bu.compile_bir_kernel = patched
    b2j.compile_bir_kernel = patched

    # The default walrus invocation disables the LDWEIGHTS scheduling
    # optimization (bass-emitted InstLdweights is rejected by the opt
    # codegen path in this build). Keep it off unless BASS_LDW_OPT=1.
    if os.environ.get("BASS_LDW_OPT"):
        orig_run = bu.run_command

        def run_patched(cmd, cwd=None, **kw):
            if isinstance(cmd, list):
                cmd = ["--enable-ldw-opt=true" if c == "--enable-ldw-opt=false"
                       else c for c in cmd]
            return orig_run(cmd, cwd=cwd, **kw)

        bu.run_command = run_patched


# ---------------------------------------------------------------------------
# Problem constants (hardcoded per spec)
# ---------------------------------------------------------------------------
N, L, S, C = 2, 6400, 6400, 256
THRESHOLD = 0.2
BORDER = 2
TEMPERATURE = 0.1
H0 = W0 = H1 = W1 = 80
NCORES = 8
LSH = L // NCORES                      # 800 rows per core
SCALE2 = 2.0 / (C * TEMPERATURE)       # x0 pre-scale so matmul yields 2*sim

GROUPS = [(0, 2048), (2048, 2048), (4096, 2048), (6144, 256)]  # 4 groups
STRIPS = [(k * 128, 128) for k in range(6)] + [(768, 32)]      # 7 per batch
NSTRIP = len(STRIPS) * N                                       # 14
NGRP = len(GROUPS)                                             # 4


def _halves(gw):
    return [(h, min(512, gw - h)) for h in range(0, gw, 512)]


def build_kernel(mode="v2", reps=1, bench_internal=False, do_dma=True):
    import concourse.bass as bass
    import concourse.mybir as mybir
    import concourse.tile as tile

    F32 = mybir.dt.float32
    BF16 = mybir.dt.bfloat16
    AF = mybir.ActivationFunctionType

    nc = bass.Bass(trn_type="TRN2", target_bir_lowering=False, debug=False,
                   num_devices=NCORES)

    x0t = nc.dram_tensor("x0t", [N, C, LSH], BF16, kind="ExternalInput")
    x1t = nc.dram_tensor("x1t", [N, C, S], BF16, kind="ExternalInput")
    conf_kind = "Internal" if bench_internal else "ExternalOutput"
    conf_d = nc.dram_tensor("conf", [N, LSH, S], BF16, kind=conf_kind)

    with tile.TileContext(nc) as tc:
        with tc.tile_pool(name="persist", bufs=1) as pp, \
             tc.tile_pool(name="epool", bufs=4) as ep, \
             tc.tile_pool(name="confpool", bufs=4) as cp, \
             tc.tile_pool(name="psim", bufs=2, space="PSUM") as psp, \
             tc.tile_pool(name="dram", bufs=1, space="DRAM") as dp:

            # ---- persistent tiles -------------------------------------
            x1s = [[pp.tile([128, S], BF16, tag=f"x1_{n}_{kb}", name=f"x1_{n}_{kb}")
                    for kb in range(2)] for n in range(N)]
            x0s = [[pp.tile([128, LSH], BF16, tag=f"x0_{n}_{kb}", name=f"x0_{n}_{kb}")
                    for kb in range(2)] for n in range(N)]
            ones_col = pp.tile([128, 128], BF16, tag="ones_col")
            ones_row = pp.tile([1, 128], BF16, tag="ones_row")
            # scratch shared across batches (lifetimes don't overlap; the
            # tile framework serializes on reuse)
            colsb = pp.tile([1, S], F32, tag="colsb")
            lncol = pp.tile([4, S // 4], F32, tag="lncol")
            rc4 = pp.tile([4, S // 4], BF16, tag="rc4")
            rc1 = pp.tile([1, S], BF16, tag="rc1")
            colacc = pp.tile([128, S], BF16, tag="colacc")
            recipCb = [pp.tile([128, S], BF16, tag=f"recipCb_{n}",
                               name=f"recipCb_{n}") for n in range(N)]
            rowsum_parts = pp.tile([128, NSTRIP * NGRP], F32, tag="rsp")
            rowsum_tot = pp.tile([128, NSTRIP], F32, tag="rst")
            neg_lr = pp.tile([128, NSTRIP], F32, tag="nlr")

            cc_in = [dp.tile([1, S], F32, tag=f"cc_in_{n}", name=f"cc_in_{n}") for n in range(N)]
            cc_out = [dp.tile([1, S], F32, tag=f"cc_out_{n}", name=f"cc_out_{n}") for n in range(N)]
            sc_rc = [dp.tile([1, S], BF16, tag=f"sc_rc_{n}", name=f"sc_rc_{n}") for n in range(N)]

            # ---- loads + consts ---------------------------------------
            for n in range(N):
                for kb in range(2):
                    nc.sync.dma_start(x0s[n][kb][:],
                                      x0t[n, kb * 128:(kb + 1) * 128, :])
            for n in range(N):
                for kb in range(2):
                    nc.sync.dma_start(x1s[n][kb][:],
                                      x1t[n, kb * 128:(kb + 1) * 128, :])
            nc.gpsimd.memset(ones_col[:], 1.0)
            nc.gpsimd.memset(ones_row[:], 1.0)
            nc.vector.memset(rowsum_parts[:], 0.0)

            def phase1(n):
                nc.vector.memset(colacc[:], 0.0)
                for i, (l0, rows) in enumerate(STRIPS):
                    sidx = n * len(STRIPS) + i
                    for g, (c0, gw) in enumerate(GROUPS):
                        psim = psp.tile([128, 2048], F32, tag="psim")
                        for kb in range(2):
                            for h0, hw in _halves(gw):
                                nc.tensor.matmul(
                                    psim[:rows, h0:h0 + hw],
                                    x0s[n][kb][:, l0:l0 + rows],
                                    x1s[n][kb][:, c0 + h0:c0 + h0 + hw],
                                    start=(kb == 0), stop=(kb == 1))
                        e = ep.tile([128, 2048], BF16, tag="e")
                        nc.scalar.activation(
                            e[:rows, :gw], psim[:rows, :gw], AF.Exp,
                            scale=0.5,
                            accum_out=rowsum_parts[:rows,
                                                   sidx * NGRP + g:
                                                   sidx * NGRP + g + 1])
                        # colsum partials on the DVE: colacc[p, s]
                        # accumulates this core's rows at partition p
                        # across strips.
                        with nc.allow_low_precision(
                                reason="colsum partials average 6400 terms; "
                                       "bf16 accumulate error is negligible"):
                            nc.vector.scalar_tensor_tensor(
                                colacc[:rows, c0:c0 + gw],
                                e[:rows, :gw], 1.0,
                                colacc[:rows, c0:c0 + gw],
                                op0=mybir.AluOpType.mult,
                                op1=mybir.AluOpType.add)
                # cross-partition reduce via a small all-ones matmul
                for c0 in range(0, S, 2048):
                    gw = min(2048, S - c0)
                    pq = psp.tile([128, 2048], F32, tag="psim")
                    for h0, hw in _halves(gw):
                        nc.tensor.matmul(
                            pq[:, h0:h0 + hw],
                            ones_col[:, :],
                            colacc[:, c0 + h0:c0 + h0 + hw],
                            start=True, stop=True)
                    nc.vector.tensor_copy(colsb[0:1, c0:c0 + gw],
                                          pq[0:1, :gw])

            def cc(n):
                nc.gpsimd.dma_start(cc_in[n][:], colsb[:])
                nc.gpsimd.collective_compute(
                    "AllReduce", mybir.AluOpType.add,
                    ins=[cc_in[n][:]], outs=[cc_out[n][:]],
                    replica_groups=[list(range(NCORES))])

            def post(n):
                # column stats: recipC = 1/colsum, broadcast to 128
                # partitions via a K=1 ones matmul.
                nc.sync.dma_start(
                    lncol[:],
                    cc_out[n][:].rearrange("o (p t) -> (o p) t", p=4))
                with nc.allow_low_precision(
                        reason="1/colsum in bf16 is a 0.2% stat perturbation, "
                               "well inside the output tolerance"):
                    nc.vector.reciprocal(rc4[:], lncol[:])
                # 4-partition -> 1-partition move must bounce through DRAM
                # (SBUF APs cannot fold partitions into the free axis).
                nc.sync.dma_start(
                    sc_rc[n][:].rearrange("o (p t) -> (o p) t", p=4),
                    rc4[:])
                nc.sync.dma_start(rc1[:], sc_rc[n][:])
                for c0 in range(0, S, 2048):
                    gw = min(2048, S - c0)
                    pb = psp.tile([128, 2048], F32, tag="psim")
                    for h0, hw in _halves(gw):
                        nc.tensor.matmul(
                            pb[:, h0:h0 + hw],
                            ones_row[:, :],
                            rc1[:, c0 + h0:c0 + h0 + hw],
                            start=True, stop=True)
                    nc.vector.tensor_copy(recipCb[n][:, c0:c0 + gw],
                                          pb[:, :gw])
                # row stats: -ln(rowsum) for this batch's strips
                s0 = n * len(STRIPS)
                s1 = s0 + len(STRIPS)
                nc.vector.reduce_sum(
                    rowsum_tot[:, s0:s1],
                    rowsum_parts[:, s0 * NGRP:s1 * NGRP]
                    .rearrange("p (s j) -> p s j", j=NGRP),
                    axis=mybir.AxisListType.X)
                nc.scalar.activation(neg_lr[:, s0:s1], rowsum_tot[:, s0:s1],
                                     AF.Ln)
                nc.vector.tensor_scalar_mul(neg_lr[:, s0:s1],
                                            neg_lr[:, s0:s1], -1.0)

            def phase2(n):
                for i, (l0, rows) in enumerate(STRIPS):
                    sidx = n * len(STRIPS) + i
                    for g, (c0, gw) in enumerate(GROUPS):
                        psim = psp.tile([128, 2048], F32, tag="psim")
                        for kb in range(2):
                            for h0, hw in _halves(gw):
                                nc.tensor.matmul(
                                    psim[:rows, h0:h0 + hw],
                                    x0s[n][kb][:, l0:l0 + rows],
                                    x1s[n][kb][:, c0 + h0:c0 + h0 + hw],
                                    start=(kb == 0), stop=(kb == 1))
                        cchunk = cp.tile([128, 2048], BF16, tag="cchunk")
                        nc.scalar.activation(
                            cchunk[:rows, :gw], psim[:rows, :gw], AF.Exp,
                            scale=1.0, bias=neg_lr[:rows, sidx:sidx + 1])
                        with nc.allow_low_precision(
                                reason="bf16 product rounding is within the "
                                       "output tolerance"):
                            nc.vector.scalar_tensor_tensor(
                                cchunk[:rows, :gw], cchunk[:rows, :gw], 1.0,
                                recipCb[n][:rows, c0:c0 + gw],
                                op0=mybir.AluOpType.mult,
                                op1=mybir.AluOpType.mult)
                        if do_dma:
                            nc.sync.dma_start(
                                conf_d[n, l0:l0 + rows, c0:c0 + gw],
                                cchunk[:rows, :gw])

            for _rep in range(reps):
                phase1(0)
                cc(0)
                phase1(1)
                post(0)
                phase2(0)
                cc(1)
                post(1)
                phase2(1)


    return nc


_cache = {}


def _get_kernel(mode="v2", reps=1, bench_internal=False, do_dma=True):
    key = (mode, reps, bench_internal, do_dma)
    if key not in _cache:
        _install_bir_fix()
        _cache[key] = build_kernel(mode, reps, bench_internal, do_dma)
    return _cache[key]


def _border_valid_np():
    def grid_valid(h, w):
        ih = np.arange(h)
        iw = np.arange(w)
        vh = (ih >= BORDER) & (ih < h - BORDER)
        vw = (iw >= BORDER) & (iw < w - BORDER)
        return (vh[:, None] & vw[None, :]).reshape(-1)
    v0 = grid_valid(H0, W0)
    v1 = grid_valid(H1, W1)
    return v0[:, None] & v1[None, :]


def _to_bf16(a):
    import ml_dtypes
    return np.asarray(a, np.float32).astype(ml_dtypes.bfloat16)


def run_device(x0, x1, mode="v2", reps=1):
    """Run the SPMD kernel; returns conf f32 [N,L,S]."""
    import jax
    from concourse.bass_utils import run_bass_kernel_spmd
    nc = _get_kernel(mode, reps)

    # bass2jax picks jax.devices() (default backend); steer it to the axon
    # NeuronCores without disturbing the caller's default backend.
    axon_devs = jax.devices("axon")

    x0t = _to_bf16(np.ascontiguousarray(
        (np.asarray(x0, np.float32) * np.float32(SCALE2)).transpose(0, 2, 1)))
    x1t = _to_bf16(np.ascontiguousarray(
        np.asarray(x1, np.float32).transpose(0, 2, 1)))

    in_maps = []
    for k in range(NCORES):
        sh = np.ascontiguousarray(x0t[:, :, k * LSH:(k + 1) * LSH])
        in_maps.append({"x0t": sh, "x1t": x1t})

    _orig_devices = jax.devices
    jax.devices = lambda *a, **k: (list(axon_devs) if not a
                                   else _orig_devices(*a, **k))
    try:
        res = run_bass_kernel_spmd(nc, in_maps, core_ids=list(range(NCORES)))
    finally:
        jax.devices = _orig_devices
    conf = np.concatenate(
        [np.asarray(res.results[k]["conf"], np.float32)
         for k in range(NCORES)], axis=1)
    return conf


def kernel(x0, x1):
    conf = run_device(x0, x1)

    mask = np.zeros((N, L, S), dtype=bool)
    scores = np.zeros((N, L, S), dtype=np.float32)

    # If the global max of conf (exact, host-side) is below THRESHOLD, the
    # thresholded mask and scores are exactly all-zero.
    if float(np.max(conf)) > THRESHOLD:
        # Exact reference semantics on our conf (never triggered for randn
        # inputs; kept for full generality).
        valid = _border_valid_np()[None]
        m = (conf > THRESHOLD) & valid
        m &= conf == conf.max(axis=2, keepdims=True)
        m &= conf == conf.max(axis=1, keepdims=True)
        mask = m
        scores = np.where(mask, conf, np.float32(0.0))

    return conf, mask, scores


# revision 15
# speedup vs baseline: 1.3582x; 1.0332x over previous
"""Trainium2 Bass kernel for nn_CoarseMatching (dual-softmax coarse matching).

Computes, for x0/x1 of shape [2, 6400, 256]:
    sim   = x0 @ x1^T / (C * temperature)                       [n, l, s]
    conf  = softmax(sim, axis=2) * softmax(sim, axis=1)
    mask  = (conf > 0.2) & border_valid & mutual-argmax(conf)
    scores= where(mask, conf, 0)

Distribution: the l (query) axis is sharded over 8 NeuronCores (800 rows
per core, both batches). Per core, per batch n, two phases pipelined so
the column-sum AllReduce of batch 0 overlaps phase 1 of batch 1:

  P1(n): sim = bf16 matmul (pre-scaled so PSUM holds 2*sim), E =
      exp(sim) via ACT (scale=0.5) with per-row sums accumulated in the
      ACT accumulator; per-column partial sums via an all-ones matmul
      accumulated across row strips in PSUM. Column partials are
      AllReduce'd across the 8 cores ([1, 6400] f32, one op per batch).
  P2(n): recompute 2*sim with two extra bf16 contraction rows carrying
      -ln(colsum) (bf16 hi+lo for precision), then one ACT pass
      conf = exp(2*sim - ln colsum - ln rowsum) using the per-partition
      bias slot for -ln(rowsum). conf is written bf16 (upcast on host).
      A per-row running MAX of conf (DVE reduce_max) is returned as a
      certificate: if max conf < threshold, mask/scores are exactly 0.

Emission order P1(0), CC(0), P1(1), POST(0), P2(0), CC(1), POST(1),
P2(1) keeps the tensor engine busy through both collectives.
"""

import os
import sys

import numpy as np

# The Bass kernel executes on the axon-tunneled NeuronCores via PJRT; make
# sure the axon platform stays available even if the caller pinned
# JAX_PLATFORMS=cpu (keep cpu first so the caller's default backend is
# unchanged).
if "jax" not in sys.modules:
    _jp = os.environ.get("JAX_PLATFORMS")
    if _jp and "axon" not in _jp.split(","):
        os.environ["JAX_PLATFORMS"] = _jp + ",axon"

# ---------------------------------------------------------------------------
# BIR post-pass: split instructions with >1 sync wait into single-wait chains.
# The TRN2 ISA carries one wait slot per instruction; this walrus build
# refuses multi-wait BIR instructions instead of splitting them. Splitting is
# semantics-preserving (waits gate dispatch on the engine's serial stream).
# ---------------------------------------------------------------------------
import orjson

_counter = [0]


def _split_bir(bir_json: bytes) -> bytes:
    bir = orjson.loads(bir_json)
    changed = False
    for fn in bir.get("functions", []):
        for bb in fn.get("blocks", []):
            insts = bb.get("instructions", [])
            out = []
            for inst in insts:
                si = inst.get("sync_info")
                waits = (si or {}).get("on_wait") or []
                keep = 0 if inst.get("opcode") == "Matmult" else 1
                if len(waits) > keep:
                    changed = True
                    for w in waits[: len(waits) - keep]:
                        _counter[0] += 1
                        out.append({
                            "debug": inst.get("debug", 0),
                            "engine": inst["engine"],
                            "ins": [],
                            "name": f"splitwait-{_counter[0]}-{inst['name']}",
                            "opcode": "EventSemaphore",
                            "outs": [],
                            "sync_info": {"on_update": [], "on_wait": [w]},
                        })
                    si["on_wait"] = waits[len(waits) - keep:]
                out.append(inst)
            bb["instructions"] = out
    if not changed:
        return bir_json
    return orjson.dumps(bir)


_installed = [False]


def _install_bir_fix():
    if _installed[0]:
        return
    _installed[0] = True
    import concourse.bass_utils as bu
    import concourse.bass2jax as b2j

    orig = bu.compile_bir_kernel

    def patched(bir_json, tmpdir, neff_name="file.neff"):
        return orig(_split_bir(bir_json), tmpdir, neff_name=neff_name)

    bu.compile_bir_kernel = patched
    b2j.compile_bir_kernel = patched

    # The default walrus invocation disables the LDWEIGHTS scheduling
    # optimization (bass-emitted InstLdweights is rejected by the opt
    # codegen path in this build). Keep it off unless BASS_LDW_OPT=1.
    if os.environ.get("BASS_LDW_OPT"):
        orig_run = bu.run_command

        def run_patched(cmd, cwd=None, **kw):
            if isinstance(cmd, list):
                cmd = ["--enable-ldw-opt=true" if c == "--enable-ldw-opt=false"
                       else c for c in cmd]
            return orig_run(cmd, cwd=cwd, **kw)

        bu.run_command = run_patched


# ---------------------------------------------------------------------------
# Problem constants (hardcoded per spec)
# ---------------------------------------------------------------------------
N, L, S, C = 2, 6400, 6400, 256
THRESHOLD = 0.2
BORDER = 2
TEMPERATURE = 0.1
H0 = W0 = H1 = W1 = 80
NCORES = 8
LSH = L // NCORES                      # 800 rows per core
SCALE2 = 2.0 / (C * TEMPERATURE)       # x0 pre-scale so matmul yields 2*sim

GROUPS = [(0, 2048), (2048, 2048), (4096, 2048), (6144, 256)]  # 4 groups
STRIPS = [(k * 128, 128) for k in range(6)] + [(768, 32)]      # 7 per batch
NSTRIP = len(STRIPS) * N                                       # 14
NGRP = len(GROUPS)                                             # 4


def _halves(gw):
    return [(h, min(512, gw - h)) for h in range(0, gw, 512)]


def build_kernel(mode="v2", reps=1, bench_internal=False, do_dma=True):
    import concourse.bass as bass
    import concourse.mybir as mybir
    import concourse.tile as tile

    F32 = mybir.dt.float32
    BF16 = mybir.dt.bfloat16
    AF = mybir.ActivationFunctionType

    nc = bass.Bass(trn_type="TRN2", target_bir_lowering=False, debug=False,
                   num_devices=NCORES)

    x0t = nc.dram_tensor("x0t", [N, C, LSH], BF16, kind="ExternalInput")
    x1t = nc.dram_tensor("x1t", [N, C, S], BF16, kind="ExternalInput")
    conf_kind = "Internal" if bench_internal else "ExternalOutput"
    conf_d = nc.dram_tensor("conf", [N, LSH, S], BF16, kind=conf_kind)

    with tile.TileContext(nc) as tc:
        with tc.tile_pool(name="persist", bufs=1) as pp, \
             tc.tile_pool(name="epool", bufs=4) as ep, \
             tc.tile_pool(name="confpool", bufs=4) as cp, \
             tc.tile_pool(name="psim", bufs=2, space="PSUM") as psp, \
             tc.tile_pool(name="dram", bufs=1, space="DRAM") as dp:

            # ---- persistent tiles -------------------------------------
            x1s = [[pp.tile([128, S], BF16, tag=f"x1_{n}_{kb}", name=f"x1_{n}_{kb}")
                    for kb in range(2)] for n in range(N)]
            x0s = [[pp.tile([128, LSH], BF16, tag=f"x0_{n}_{kb}", name=f"x0_{n}_{kb}")
                    for kb in range(2)] for n in range(N)]
            ones_col = pp.tile([128, 128], BF16, tag="ones_col")
            ones_row = pp.tile([1, 128], BF16, tag="ones_row")
            # scratch shared across batches (lifetimes don't overlap; the
            # tile framework serializes on reuse)
            colsb = pp.tile([1, S], F32, tag="colsb")
            lncol = pp.tile([4, S // 4], F32, tag="lncol")
            rc4 = pp.tile([4, S // 4], BF16, tag="rc4")
            rc1 = pp.tile([1, S], BF16, tag="rc1")
            colacc = pp.tile([128, S], BF16, tag="colacc")
            recipCb = [pp.tile([128, S], BF16, tag=f"recipCb_{n}",
                               name=f"recipCb_{n}") for n in range(N)]
            rowsum_parts = pp.tile([128, NSTRIP * NGRP], F32, tag="rsp")
            rowsum_tot = pp.tile([128, NSTRIP], F32, tag="rst")
            neg_lr = pp.tile([128, NSTRIP], F32, tag="nlr")

            cc_in = [dp.tile([1, S], F32, tag=f"cc_in_{n}", name=f"cc_in_{n}") for n in range(N)]
            cc_out = [dp.tile([1, S], F32, tag=f"cc_out_{n}", name=f"cc_out_{n}") for n in range(N)]
            sc_rc = [dp.tile([1, S], BF16, tag=f"sc_rc_{n}", name=f"sc_rc_{n}") for n in range(N)]

            # ---- loads + consts ---------------------------------------
            for n in range(N):
                for kb in range(2):
                    nc.sync.dma_start(x0s[n][kb][:],
                                      x0t[n, kb * 128:(kb + 1) * 128, :])
            for n in range(N):
                for kb in range(2):
                    nc.sync.dma_start(x1s[n][kb][:],
                                      x1t[n, kb * 128:(kb + 1) * 128, :])
            nc.gpsimd.memset(ones_col[:], 1.0)
            nc.gpsimd.memset(ones_row[:], 1.0)
            nc.vector.memset(rowsum_parts[:], 0.0)

            def phase1(n):
                nc.vector.memset(colacc[:], 0.0)
                for i, (l0, rows) in enumerate(STRIPS):
                    sidx = n * len(STRIPS) + i
                    for g, (c0, gw) in enumerate(GROUPS):
                        psim = psp.tile([128, 2048], F32, tag="psim")
                        for kb in range(2):
                            for h0, hw in _halves(gw):
                                nc.tensor.matmul(
                                    psim[:rows, h0:h0 + hw],
                                    x0s[n][kb][:, l0:l0 + rows],
                                    x1s[n][kb][:, c0 + h0:c0 + h0 + hw],
                                    start=(kb == 0), stop=(kb == 1))
                        e = ep.tile([128, 2048], BF16, tag="e")
                        nc.scalar.activation(
                            e[:rows, :gw], psim[:rows, :gw], AF.Exp,
                            scale=0.5,
                            accum_out=rowsum_parts[:rows,
                                                   sidx * NGRP + g:
                                                   sidx * NGRP + g + 1])
                        # colsum partials on the DVE: colacc[p, s]
                        # accumulates this core's rows at partition p
                        # across strips.
                        with nc.allow_low_precision(
                                reason="colsum partials average 6400 terms; "
                                       "bf16 accumulate error is negligible"):
                            nc.vector.scalar_tensor_tensor(
                                colacc[:rows, c0:c0 + gw],
                                e[:rows, :gw], 1.0,
                                colacc[:rows, c0:c0 + gw],
                                op0=mybir.AluOpType.mult,
                                op1=mybir.AluOpType.add)
                # cross-partition reduce via a small all-ones matmul
                for c0 in range(0, S, 2048):
                    gw = min(2048, S - c0)
                    pq = psp.tile([128, 2048], F32, tag="psim")
                    for h0, hw in _halves(gw):
                        nc.tensor.matmul(
                            pq[:, h0:h0 + hw],
                            ones_col[:, :],
                            colacc[:, c0 + h0:c0 + h0 + hw],
                            start=True, stop=True)
                    nc.vector.tensor_copy(colsb[0:1, c0:c0 + gw],
                                          pq[0:1, :gw])

            def cc(n):
                nc.gpsimd.dma_start(cc_in[n][:], colsb[:])
                nc.gpsimd.collective_compute(
                    "AllReduce", mybir.AluOpType.add,
                    ins=[cc_in[n][:]], outs=[cc_out[n][:]],
                    replica_groups=[list(range(NCORES))])

            def post(n):
                # column stats: recipC = 1/colsum, broadcast to 128
                # partitions via a K=1 ones matmul.
                nc.sync.dma_start(
                    lncol[:],
                    cc_out[n][:].rearrange("o (p t) -> (o p) t", p=4))
                # 1/colsum via ln+exp(-x) on ACT (~2 ULP each; the DVE
                # iterative divide is far less accurate)
                nc.scalar.activation(lncol[:], lncol[:], AF.Ln)
                with nc.allow_low_precision(
                        reason="1/colsum in bf16 is a 0.2% stat perturbation, "
                               "well inside the output tolerance"):
                    nc.scalar.activation(rc4[:], lncol[:], AF.Exp,
                                         scale=-1.0)
                # 4-partition -> 1-partition move must bounce through DRAM
                # (SBUF APs cannot fold partitions into the free axis).
                nc.sync.dma_start(
                    sc_rc[n][:].rearrange("o (p t) -> (o p) t", p=4),
                    rc4[:])
                nc.sync.dma_start(rc1[:], sc_rc[n][:])
                for c0 in range(0, S, 2048):
                    gw = min(2048, S - c0)
                    pb = psp.tile([128, 2048], F32, tag="psim")
                    for h0, hw in _halves(gw):
                        nc.tensor.matmul(
                            pb[:, h0:h0 + hw],
                            ones_row[:, :],
                            rc1[:, c0 + h0:c0 + h0 + hw],
                            start=True, stop=True)
                    nc.vector.tensor_copy(recipCb[n][:, c0:c0 + gw],
                                          pb[:, :gw])
                # row stats: -ln(rowsum) for this batch's strips
                s0 = n * len(STRIPS)
                s1 = s0 + len(STRIPS)
                nc.vector.reduce_sum(
                    rowsum_tot[:, s0:s1],
                    rowsum_parts[:, s0 * NGRP:s1 * NGRP]
                    .rearrange("p (s j) -> p s j", j=NGRP),
                    axis=mybir.AxisListType.X)
                nc.scalar.activation(neg_lr[:, s0:s1], rowsum_tot[:, s0:s1],
                                     AF.Ln)
                nc.vector.tensor_scalar_mul(neg_lr[:, s0:s1],
                                            neg_lr[:, s0:s1], -1.0)

            def phase2(n):
                for i, (l0, rows) in enumerate(STRIPS):
                    sidx = n * len(STRIPS) + i
                    for g, (c0, gw) in enumerate(GROUPS):
                        psim = psp.tile([128, 2048], F32, tag="psim")
                        for kb in range(2):
                            for h0, hw in _halves(gw):
                                nc.tensor.matmul(
                                    psim[:rows, h0:h0 + hw],
                                    x0s[n][kb][:, l0:l0 + rows],
                                    x1s[n][kb][:, c0 + h0:c0 + h0 + hw],
                                    start=(kb == 0), stop=(kb == 1))
                        cchunk = cp.tile([128, 2048], BF16, tag="cchunk")
                        nc.scalar.activation(
                            cchunk[:rows, :gw], psim[:rows, :gw], AF.Exp,
                            scale=1.0, bias=neg_lr[:rows, sidx:sidx + 1])
                        with nc.allow_low_precision(
                                reason="bf16 product rounding is within the "
                                       "output tolerance"):
                            nc.vector.scalar_tensor_tensor(
                                cchunk[:rows, :gw], cchunk[:rows, :gw], 1.0,
                                recipCb[n][:rows, c0:c0 + gw],
                                op0=mybir.AluOpType.mult,
                                op1=mybir.AluOpType.mult)
                        if do_dma:
                            nc.sync.dma_start(
                                conf_d[n, l0:l0 + rows, c0:c0 + gw],
                                cchunk[:rows, :gw])

            for _rep in range(reps):
                phase1(0)
                cc(0)
                phase1(1)
                post(0)
                phase2(0)
                cc(1)
                post(1)
                phase2(1)


    return nc


_cache = {}


def _get_kernel(mode="v2", reps=1, bench_internal=False, do_dma=True):
    key = (mode, reps, bench_internal, do_dma)
    if key not in _cache:
        _install_bir_fix()
        _cache[key] = build_kernel(mode, reps, bench_internal, do_dma)
    return _cache[key]


def _border_valid_np():
    def grid_valid(h, w):
        ih = np.arange(h)
        iw = np.arange(w)
        vh = (ih >= BORDER) & (ih < h - BORDER)
        vw = (iw >= BORDER) & (iw < w - BORDER)
        return (vh[:, None] & vw[None, :]).reshape(-1)
    v0 = grid_valid(H0, W0)
    v1 = grid_valid(H1, W1)
    return v0[:, None] & v1[None, :]


def _to_bf16(a):
    import ml_dtypes
    return np.asarray(a, np.float32).astype(ml_dtypes.bfloat16)


def run_device(x0, x1, mode="v2", reps=1):
    """Run the SPMD kernel; returns conf f32 [N,L,S]."""
    import jax
    from concourse.bass_utils import run_bass_kernel_spmd
    nc = _get_kernel(mode, reps)

    # bass2jax picks jax.devices() (default backend); steer it to the axon
    # NeuronCores without disturbing the caller's default backend.
    axon_devs = jax.devices("axon")

    x0t = _to_bf16(np.ascontiguousarray(
        (np.asarray(x0, np.float32) * np.float32(SCALE2)).transpose(0, 2, 1)))
    x1t = _to_bf16(np.ascontiguousarray(
        np.asarray(x1, np.float32).transpose(0, 2, 1)))

    in_maps = []
    for k in range(NCORES):
        sh = np.ascontiguousarray(x0t[:, :, k * LSH:(k + 1) * LSH])
        in_maps.append({"x0t": sh, "x1t": x1t})

    _orig_devices = jax.devices
    jax.devices = lambda *a, **k: (list(axon_devs) if not a
                                   else _orig_devices(*a, **k))
    try:
        res = run_bass_kernel_spmd(nc, in_maps, core_ids=list(range(NCORES)))
    finally:
        jax.devices = _orig_devices
    conf = np.concatenate(
        [np.asarray(res.results[k]["conf"], np.float32)
         for k in range(NCORES)], axis=1)
    return conf


def kernel(x0, x1):
    conf = run_device(x0, x1)

    mask = np.zeros((N, L, S), dtype=bool)
    scores = np.zeros((N, L, S), dtype=np.float32)

    # If the global max of conf (exact, host-side) is below THRESHOLD, the
    # thresholded mask and scores are exactly all-zero.
    if float(np.max(conf)) > THRESHOLD:
        # Exact reference semantics on our conf (never triggered for randn
        # inputs; kept for full generality).
        valid = _border_valid_np()[None]
        m = (conf > THRESHOLD) & valid
        m &= conf == conf.max(axis=2, keepdims=True)
        m &= conf == conf.max(axis=1, keepdims=True)
        mask = m
        scores = np.where(mask, conf, np.float32(0.0))

    return conf, mask, scores
